# revision 18
# baseline (speedup 1.0000x reference)
"""Trainium kernel for nn_CompressedSensingInception.

Strategy: pure data parallel over batch (32 -> 4 per core x 8 cores).
The 100-iteration FISTA loop (99% of FLOPs) runs on device in a
separable-PSF formulation: mat = kron(gi, gj) (rank-1 factorization
recovered from the input matrix), so each 5184x81 projection becomes
two 72x9 contractions. The tiny pre/post conv layers run on host.

Device state layout: Y stored as [72(j), 12*72(r,i)] per core, where
r = local_batch*3 + channel, and Y[r, i, j] = state[j, r*72+i].
"""

import numpy as np

import concourse.bass as bass
import concourse.bacc as bacc
import concourse.tile as tile
from concourse import mybir
from concourse.bass_utils import run_bass_kernel_spmd

B = 32
NCORES = 8
BL = B // NCORES          # local batch
R = BL * 3                # rows per core (batch x channel)
ITERS = 100
ALPHA = np.float32(0.3)
BN_SCALE = np.float32(1.0 / np.sqrt(1.0 + 1e-3))
F32 = mybir.dt.float32


# ----------------------------------------------------------------- host math
def _leaky(x):
    return np.where(x >= 0, x, ALPHA * x).astype(np.float32)


def _sigmoid(x):
    return (1.0 / (1.0 + np.exp(-x.astype(np.float64)))).astype(np.float32)


def _conv_np(x, k, strides=(1, 1)):
    """NHWC conv, SAME padding, matching lax.conv_general_dilated."""
    n, h, w, ci = x.shape
    kh, kw, _, co = k.shape
    sy, sx = strides
    oh = -(-h // sy)
    ow = -(-w // sx)
    pad_h = max((oh - 1) * sy + kh - h, 0)
    pad_w = max((ow - 1) * sx + kw - w, 0)
    plo_h, phi_h = pad_h // 2, pad_h - pad_h // 2
    plo_w, phi_w = pad_w // 2, pad_w - pad_w // 2
    xp = np.zeros((n, h + pad_h, w + pad_w, ci), np.float32)
    xp[:, plo_h:plo_h + h, plo_w:plo_w + w, :] = x
    out = np.zeros((n, oh, ow, co), np.float32)
    for dy in range(kh):
        for dx in range(kw):
            sl = xp[:, dy:dy + (oh - 1) * sy + 1:sy, dx:dx + (ow - 1) * sx + 1:sx, :]
            out += sl @ k[dy, dx]
    return out


def _factor_mat(mat):
    """mat[(i*72+j), (a*9+b)] = gi[i,a] * gj[j,b]; recover rank-1 factors."""
    X = mat.reshape(72, 72, 9, 9).transpose(0, 2, 1, 3).reshape(648, 648)
    U, S, Vt = np.linalg.svd(X.astype(np.float64))
    gi = (U[:, 0] * np.sqrt(S[0])).reshape(72, 9)
    gj = (Vt[0, :] * np.sqrt(S[0])).reshape(72, 9)
    return gi.astype(np.float32), gj.astype(np.float32)


def _momentum_coefs():
    """c_i = (t_i - 1)/t_{i+1} in f32 arithmetic (matches reference)."""
    cs = []
    t = np.float32(1.0)
    for _ in range(ITERS):
        t_n = np.float32((np.float32(1.0) + np.float32(np.sqrt(np.float32(1.0) + np.float32(4.0) * t * t))) / np.float32(2.0))
        cs.append(float((t - np.float32(1.0)) / t_n))
        t = t_n
    return cs


# ------------------------------------------------------------- device kernel
_CACHED_NC = None
_TRACE_SIM = False


def _build_nc():
    global _CACHED_NC
    if _CACHED_NC is not None:
        return _CACHED_NC
    nc = bacc.Bacc("TRN2", target_bir_lowering=False, debug=False,
                   num_devices=NCORES)
    W = R * 72  # 864
    im9 = nc.dram_tensor("im9", [9, R * 9], F32, kind="ExternalInput")
    gj_d = nc.dram_tensor("gj", [72, 9], F32, kind="ExternalInput")
    gi_d = nc.dram_tensor("gi", [72, 9], F32, kind="ExternalInput")
    giT_d = nc.dram_tensor("giT", [9, 72], F32, kind="ExternalInput")
    gjbd0_d = nc.dram_tensor("gjbd0", [54, W // 2], F32, kind="ExternalInput")
    gjbd1_d = nc.dram_tensor("gjbd1", [54, W // 2], F32, kind="ExternalInput")
    id_d = nc.dram_tensor("ident", [72, 72], F32, kind="ExternalInput")
    yout = nc.dram_tensor("yout", [72, W], F32, kind="ExternalOutput")

    cs = _momentum_coefs()
    H = W // 2  # 432, psum-bank half

    with tile.TileContext(nc, trace_sim=_TRACE_SIM) as tc:
        with tc.tile_pool(name="const", bufs=1) as cp, \
             tc.tile_pool(name="state", bufs=1) as sp, \
             tc.tile_pool(name="tmp", bufs=2) as tp, \
             tc.tile_pool(name="ps", bufs=1, space="PSUM") as pp:
            # constants
            c_im = cp.tile([9, R * 9], F32)
            c_gj = cp.tile([72, 9], F32)
            c_gi = cp.tile([72, 9], F32)
            c_giT = cp.tile([9, 72], F32)
            c_gjbd0 = cp.tile([54, H], F32)
            c_gjbd1 = cp.tile([54, H], F32)
            c_id = cp.tile([72, 72], F32)
            for t_, d_ in ((c_im, im9), (c_gj, gj_d), (c_gi, gi_d),
                           (c_giT, giT_d), (c_gjbd0, gjbd0_d),
                           (c_gjbd1, gjbd1_d), (c_id, id_d)):
                nc.sync.dma_start(out=t_[:], in_=d_[:])
            c_gjbd = [c_gjbd0, c_gjbd1]

            # rotating state tiles
            st = [sp.tile([72, W], F32, tag=f"s{i}", name=f"s{i}")
                  for i in range(4)]
            for s in st[:2]:
                nc.vector.memset(s[:], 0.0)
            yt, yl = st[0], st[1]
            free0, free1 = st[2], st[3]

            for it in range(ITERS):
                c = cs[it]
                last = it == ITERS - 1
                yn, yx = free0, free1

                # two half-pipelines (r 0-5 | r 6-11) so matmul and
                # elementwise stages of opposite halves overlap
                Tps = pp.tile([72, R * 9], F32, tag="Tps")
                Rps = [pp.tile([72, H], F32, tag="Rps0", name="Rps0"),
                       pp.tile([72, H], F32, tag="Rps1", name="Rps1")]
                for h in range(2):
                    sl = slice(h * H, (h + 1) * H)
                    s9 = slice(h * 54, (h + 1) * 54)
                    # A: T[i,(r,b)] = sum_j Y[j,(r,i)] gj[j,b]
                    for o in range(6):
                        r = h * 6 + o
                        nc.tensor.matmul(Tps[:, r * 9:(r + 1) * 9],
                                         yt[:, r * 72:(r + 1) * 72], c_gj[:],
                                         start=True, stop=True)
                    Tsb = tp.tile([72, 54], F32, tag=f"Tsb{h}", name=f"Tsb{h}")
                    nc.scalar.activation(Tsb[:], Tps[:, s9],
                                         mybir.ActivationFunctionType.Copy)
                    # B: u[a,(r,b)] = sum_i gi[i,a] T[i,(r,b)]
                    ups = pp.tile([9, 54], F32, tag=f"ups{h}", name=f"ups{h}")
                    nc.tensor.matmul(ups[:], c_gi[:], Tsb[:], start=True, stop=True)
                    # resid = im - u  (im pre-scaled by 1/lam on host; lam==1)
                    resid = tp.tile([9, 54], F32, tag=f"res{h}", name=f"res{h}")
                    nc.vector.tensor_tensor(resid[:], c_im[:, s9], ups[:],
                                            mybir.AluOpType.subtract)
                    # C': P'[(r,b), i] = sum_a resid[a,(r,b)] giT[a,i]
                    PTps = pp.tile([54, 72], F32, tag=f"PTps{h}", name=f"PTps{h}")
                    nc.tensor.matmul(PTps[:], resid[:], c_giT[:],
                                     start=True, stop=True)
                    PTsb = tp.tile([54, 72], F32, tag=f"PTsb{h}", name=f"PTsb{h}")
                    nc.scalar.activation(PTsb[:], PTps[:],
                                         mybir.ActivationFunctionType.Copy)
                    # D: wv[j,(r,i)] = Y + sum_b gj[j,b] P'[(r,b), i]
                    nc.tensor.matmul(Rps[h][:], c_id[:], yt[:, sl],
                                     start=True, stop=False)
                    for o in range(6):
                        nc.tensor.matmul(Rps[h][:, o * 72:(o + 1) * 72],
                                         c_gjbd[h][:, o * 72:(o + 1) * 72],
                                         PTsb[:], start=False, stop=(o == 5),
                                         skip_group_check=True)
                    # yn = wv - clamp(wv, -1, 1)   (soft threshold, lam==1)
                    t1 = tp.tile([72, H], F32, tag=f"t1{h}", name=f"t1{h}")
                    nc.vector.tensor_scalar(out=t1[:], in0=Rps[h][:],
                                            scalar1=1.0, scalar2=-1.0,
                                            op0=mybir.AluOpType.min,
                                            op1=mybir.AluOpType.max)
                    nc.vector.tensor_tensor(yn[:, sl], Rps[h][:], t1[:],
                                            mybir.AluOpType.subtract)
                    if not last:
                        # yx = yn + c*(yn - yl)
                        d_ = tp.tile([72, H], F32, tag=f"d{h}", name=f"d{h}")
                        e_ = tp.tile([72, H], F32, tag=f"e{h}", name=f"e{h}")
                        nc.gpsimd.tensor_tensor(d_[:], yn[:, sl], yl[:, sl],
                                                mybir.AluOpType.subtract)
                        nc.scalar.activation(e_[:], d_[:],
                                             mybir.ActivationFunctionType.Copy,
                                             scale=float(c))
                        nc.gpsimd.tensor_tensor(yx[:, sl], yn[:, sl], e_[:],
                                                mybir.AluOpType.add)
                if last:
                    nc.sync.dma_start(out=yout[:], in_=yn[:])
                else:
                    free0, free1 = yt, yl
                    yt, yl = yx, yn
    nc.compile()
    _CACHED_NC = nc
    return nc


# ------------------------------------------------------------------ frontend
def kernel(inp, mat, w1_k, w1_b, x1_k, x1_b, c51_k, c51_b, c15_k, c15_b,
           c55_k, c55_b, x2_k, x2_b, y17_k, y17_b, y71_k, y71_b, yc_k, yc_b,
           d1_k, d2_k, h1_w, h1_b, h2_w, h2_b, h3_w, h3_b):
    inp = np.asarray(inp, np.float32)
    mat = np.asarray(mat, np.float32)

    # ---- host: w branch
    w = _leaky(inp @ np.asarray(w1_k)[0, 0] + np.asarray(w1_b))

    # ---- host: z branch -> lam
    z = inp * BN_SCALE
    z = _leaky(_conv_np(z, np.asarray(d1_k), (3, 3)) * BN_SCALE)
    z = _leaky(_conv_np(z, np.asarray(d2_k), (3, 3)) * BN_SCALE)
    z = z.reshape(B, -1)
    z = z @ np.asarray(h1_w) + np.asarray(h1_b)
    z = z @ np.asarray(h2_w) + np.asarray(h2_b)
    z = z @ np.asarray(h3_w) + np.asarray(h3_b)
    param = np.float32(0.1) * _sigmoid(z)          # (B,1)
    lam = (param * np.float32(0.1)).astype(np.float32)  # (B,1)

    # ---- host: x branch front
    xn = inp / (np.float32(0.001) + np.max(np.abs(inp), axis=1, keepdims=True))
    x = _leaky(xn @ np.asarray(x1_k)[0, 0] + np.asarray(x1_b))  # (B,9,9,3)
    im = x.reshape(B, 81, 3).transpose(0, 2, 1).astype(np.float32)  # (B,3,81)

    # ---- device: FISTA
    gi, gj = _factor_mat(mat)
    nc = _build_nc()
    in_maps = []
    for ci in range(NCORES):
        b0 = ci * BL
        im_l = im[b0:b0 + BL].reshape(R, 81)          # r = bl*3 + ch
        lam_l = np.repeat(lam[b0:b0 + BL, 0], 3).astype(np.float32)  # (R,)
        im_l = (im_l / lam_l[:, None]).astype(np.float32)  # lam-normalized
        im9 = np.ascontiguousarray(
            im_l.reshape(R, 9, 9).transpose(1, 0, 2).reshape(9, R * 9))
        gjbd_h = np.zeros((2, 54, 6 * 72), np.float32)
        for r in range(R):
            h, o = divmod(r, 6)
            gjbd_h[h, 9 * o:9 * o + 9, 72 * o:72 * o + 72] = gj.T
        in_maps.append({
            "im9": im9,
            "gj": gj, "gi": gi,
            "giT": np.ascontiguousarray(gi.T),
            "gjbd0": gjbd_h[0], "gjbd1": gjbd_h[1],
            "ident": np.eye(72, dtype=np.float32),
        })
    global _last_in_maps
    _last_in_maps = in_maps
    res = run_bass_kernel_spmd(nc, in_maps, list(range(NCORES)))

    y_new = np.zeros((B, 3, 5184), np.float32)
    for ci in range(NCORES):
        yo = res.results[ci]["yout"]                  # [72(j), (r,i)]
        lam_l = np.repeat(lam[ci * BL:(ci + 1) * BL, 0], 3).astype(np.float32)
        yl = yo.reshape(72, R, 72).transpose(1, 2, 0) * lam_l[:, None, None]
        y_new[ci * BL:(ci + 1) * BL] = yl.reshape(BL, 3, 5184).astype(np.float32)

    cs_out = y_new.transpose(0, 2, 1).reshape(B, 72, 72, 3)

    # ---- host: post convs
    x = _conv_np(cs_out, np.asarray(c51_k), (2, 2)) + np.asarray(c51_b)
    x = _conv_np(x, np.asarray(c15_k), (2, 2)) + np.asarray(c15_b)
    x = _conv_np(x, np.asarray(c55_k), (2, 2)) + np.asarray(c55_b)
    x = _leaky(x @ np.asarray(x2_k)[0, 0] + np.asarray(x2_b))   # (B,9,9,8)

    # ---- host: y branch
    y1 = _conv_np(inp, np.asarray(y17_k)) + np.asarray(y17_b)
    y2 = _conv_np(inp, np.asarray(y71_k)) + np.asarray(y71_b)
    ycat = np.concatenate([y1, y2], axis=-1)
    y = _leaky(ycat @ np.asarray(yc_k)[0, 0] + np.asarray(yc_b))

    out = np.concatenate([w, x, y], axis=-1).astype(np.float32)
    return out, cs_out
